# revision 26
# baseline (speedup 1.0000x reference)
"""Trainium2 Bass kernel for LoRA-augmented causal attention (GPT2-style block).

Problem: x[2,2048,768] -> qkv = x@W_attn + b + 16*(x@A_c^T)@B_c^T (3 chunks),
causal softmax attention (12 heads, dh=64), output proj, returns
(a[2,2048,768], present[2,2,12,2048,64]).

Sharding: 8 cores = 2 batches x 4 head-groups (3 heads each). Each core:
  - computes its 576 columns of qkv in transposed layout qkvT[576, 2048] from
    xT[768, 2048] (host-pretransposed). Row order is interleaved as
    [q0 q1 | q2 v0 | k0 k1 | k2 v1 | v2] so that each q_h and k_h slice sits
    at the same SBUF base partition (PE array-row wiring requires matmul
    operands to share base partition) without any padding rows.
    LoRA is folded into the weights on host (W_eff = W + 16*(B_c A_c)^T) and
    the q columns are pre-scaled by 1/sqrt(dh)=0.125 (exact power of two),
  - PE-transposes k,v back to natural layout for the `present` output and for
    the o = p @ v matmul (v augmented with a ones column so the softmax
    denominator Z falls out of the same matmul),
  - causal attention per head in [sk,sq] layout (no max subtraction needed:
    |scores| <= ~5 for this distribution); sk-tiles processed in pairs with
    one [128,1024] exp per pair,
  - partial output projection aT_part[768, 2048] = Wp_slice^T @ oT.
Host reduces the 4 partial aT per batch, adds b_proj, and assembles present.

Matmuls run as float32r (HW-rounded 20-bit fp32); softmax normalization
stays fp32.
"""

import numpy as np
from contextlib import ExitStack

import concourse.bass as bass
import concourse.mybir as mybir
import concourse.tile as tile
from concourse import bacc
from concourse import bass_utils
from concourse.masks import make_identity

F32 = mybir.dt.float32
F32R = mybir.dt.float32r

S = 2048           # sequence length
NX = 768           # model dim
DH = 64            # head dim
HPC = 3            # heads per core
QCP = 576          # qkvT rows (no padding; interleaved layout)
NK = NX // 128     # 6 k-tiles over the model dim
NCHUNK = 4         # sq chunks of 512
SQC = 512
STILES = S // 128  # 16

# qkvT row-tiles: (row0, nrows); rows = [q0 q1 | v0 q2 | k0 k1 | v1 k2 | v2]
MT = [(0, 128), (128, 128), (256, 128), (384, 128), (512, 64)]
# (tile, base) for each logical 64-row slice
Q_AT = [(0, 0), (0, 64), (1, 64)]
K_AT = [(2, 0), (2, 64), (3, 64)]
V_AT = [(1, 0), (3, 0), (4, 0)]


class _nullcontext:
    def __enter__(self):
        return None

    def __exit__(self, *a):
        return False


def round_f32r(a):
    """Round fp32 array to the fp32r grid (top 20 bits, RNE)."""
    arr = np.ascontiguousarray(a, dtype=np.float32).copy()
    u = arr.view(np.uint32)
    lsb = (u >> 12) & 1
    u += 0x7FF + lsb
    u &= np.uint32(0xFFFFF000)
    return arr


def build_nc(loop_n=1, io="external", phases="BCDE", noact=False):
    nc = bacc.Bacc("TRN2", target_bir_lowering=False, debug=False)

    kin = "ExternalInput" if io == "external" else "Internal"
    kout = "ExternalOutput" if io == "external" else "Internal"
    xt = nc.dram_tensor("xt", [NX, S], F32R, kind=kin)
    wc = nc.dram_tensor("wc", [NX, QCP], F32R, kind=kin)
    bc = nc.dram_tensor("bc", [QCP, 1], F32, kind=kin)
    wp = nc.dram_tensor("wp", [HPC * DH, NX], F32R, kind=kin)
    masks = nc.dram_tensor("masks", [2, 128, 2 * SQC], F32R, kind=kin)
    zeros = nc.dram_tensor("zeros", [64, S], F32R, kind=kin)

    k_out = nc.dram_tensor("k_out", [HPC, S, DH], F32, kind=kout)
    v_out = nc.dram_tensor("v_out", [HPC, S, DH], F32, kind=kout)
    at_out = nc.dram_tensor("at_out", [NX, S], F32, kind=kout)
    dummy = nc.dram_tensor("tick", [1, 1], F32,
                           kind="ExternalOutput") if io != "external" else None

    with tile.TileContext(nc) as tc:
        with ExitStack() as ctx:
            consts = ctx.enter_context(tc.tile_pool(name="consts", bufs=1))
            persist = ctx.enter_context(tc.tile_pool(name="persist", bufs=1))
            vpool = ctx.enter_context(tc.tile_pool(name="vsb", bufs=16))
            work = ctx.enter_context(tc.tile_pool(name="work", bufs=3))
            small = ctx.enter_context(tc.tile_pool(name="small", bufs=4))

            identf = consts.tile([128, 128], F32)
            make_identity(nc, identf[:])
            ident = consts.tile([128, 128], F32R)
            nc.vector.tensor_copy(ident[:], identf[:])
            ones3 = consts.tile([128, HPC, 1], F32)
            nc.vector.memset(ones3[:], 1.0)

            mask_sb = consts.tile([128, 2, 2 * SQC], F32R)
            wct = consts.tile([128, NK, QCP], F32R)
            bct = [consts.tile([128, 1], F32, tag=f"bct{mi}", name=f"bct{mi}")
                   for mi in range(len(MT))]
            wpt0 = consts.tile([128, NX], F32R)
            wpt1 = consts.tile([128, NX], F32R)
            xts = persist.tile([128, NK, S], F32R)
            kz = [persist.tile([128, S], F32R, tag=f"kz{h}", name=f"kz{h}")
                  for h in range(HPC)]

            qkvt = [persist.tile([nr, S], F32R, tag=f"qkvt{mi}", name=f"qkvt{mi}")
                    for mi, (r0, nr) in enumerate(MT)]

            ot_a = persist.tile([128, S], F32R, tag="ot_a")   # heads 0,1
            ot_b = persist.tile([128, S], F32R, tag="ot_b")   # head 2 (+zero pad rows)

            env = dict(
                phases=phases, noact=noact,
                xt=xt, wc=wc, bc=bc, wp=wp, masks=masks,
                k_out=k_out, v_out=v_out, at_out=at_out,
                vpool=vpool, work=work, small=small,
                ident=ident, ones3=ones3, mask_sb=mask_sb, wct=wct, bct=bct,
                wpt0=wpt0, wpt1=wpt1, xts=xts, qkvt=qkvt, kz=kz,
                zeros=zeros, ot_a=ot_a, ot_b=ot_b)

            loop_ctx = tc.For_i(0, loop_n, 1) if loop_n > 1 else _nullcontext()
            with loop_ctx:
                _build_body(nc, tc, env)
            if dummy is not None:
                dtile = consts.tile([1, 1], F32)
                nc.vector.tensor_copy(dtile[:], ones3[0:1, 0, :])
                nc.sync.dma_start(out=dummy[:], in_=dtile[:])

    nc.compile()
    return nc


def _build_body(nc, tc, env):
    xt = env["xt"]; wc = env["wc"]; bc = env["bc"]; wp = env["wp"]
    masks = env["masks"]; k_out = env["k_out"]; v_out = env["v_out"]
    at_out = env["at_out"]
    vpool = env["vpool"]; work = env["work"]; small = env["small"]
    ident = env["ident"]; ones3 = env["ones3"]; mask_sb = env["mask_sb"]
    wct = env["wct"]; bct = env["bct"]; wpt0 = env["wpt0"]; wpt1 = env["wpt1"]
    xts = env["xts"]; qkvt = env["qkvt"]; kz = env["kz"]; zeros = env["zeros"]
    ot_a = env["ot_a"]; ot_b = env["ot_b"]

    def qslice(h, col0, ncols):
        mi, off = Q_AT[h]
        return qkvt[mi][off:off + 64, col0:col0 + ncols]

    def kslice(h, col0, ncols):
        mi, off = K_AT[h]
        return qkvt[mi][off:off + 64, col0:col0 + ncols]

    # ---------------- input DMA loads ----------------
    # critical-first: wct + the n=0 column pieces of xT pace the first
    # qkv chunk; the rest streams behind
    for k in range(NK):
        nc.sync.dma_start(out=wct[:, k, :], in_=wc[k * 128:(k + 1) * 128, :])
        nc.sync.dma_start(out=xts[:, k, 0:SQC],
                          in_=xt[k * 128:(k + 1) * 128, 0:SQC])
    for n in range(1, NCHUNK):
        for k in range(NK):
            nc.sync.dma_start(
                out=xts[:, k, n * SQC:(n + 1) * SQC],
                in_=xt[k * 128:(k + 1) * 128, n * SQC:(n + 1) * SQC])
    for mi, (r0, nr) in enumerate(MT):
        nc.sync.dma_start(out=bct[mi][:nr, :], in_=bc[r0:r0 + nr, :])
    for j in range(2):
        nc.sync.dma_start(out=mask_sb[:, j, :], in_=masks[j])
    nc.sync.dma_start(out=wpt0[:], in_=wp[0:128, :])
    nc.sync.dma_start(out=wpt1[0:64, :], in_=wp[128:192, :])
    # zero-padding halves for K=128 matmuls
    nc.sync.dma_start(out=wpt1[64:128, :], in_=zeros[:, 0:NX])
    nc.sync.dma_start(out=kz[0][64:128, :], in_=zeros[:])
    nc.sync.dma_start(out=kz[1][0:64, :], in_=zeros[:])
    nc.sync.dma_start(out=kz[2][0:64, :], in_=zeros[:])
    nc.sync.dma_start(out=ot_b[64:128, :], in_=zeros[:])

    # ---------------- Phase B: qkvT = Wc^T @ xT (+bias), n-outer ----------
    with ExitStack() as s1:
        qkv_ps = s1.enter_context(
            tc.tile_pool(name="qkv_ps", bufs=1, space="PSUM"))
        tp_ps = s1.enter_context(
            tc.tile_pool(name="tp_ps", bufs=1, space="PSUM"))

        for n in range(NCHUNK):
            ps = [qkv_ps.tile([nr, SQC], F32, tag=f"qps{mi}", name=f"qps{mi}")
                  for mi, (r0, nr) in enumerate(MT)]
            for k in range(NK):
                rhs = xts[:, k, n * SQC:(n + 1) * SQC]
                for mi, (r0, nr) in enumerate(MT):
                    nc.tensor.matmul(ps[mi][:], wct[:, k, r0:r0 + nr], rhs,
                                     start=(k == 0), stop=(k == NK - 1))
            for mi, (r0, nr) in enumerate(MT):
                nc.scalar.activation(
                    out=qkvt[mi][:, n * SQC:(n + 1) * SQC], in_=ps[mi][:],
                    func=mybir.ActivationFunctionType.Identity,
                    bias=bct[mi][:nr, :], scale=1.0)
            # k heads also into zero-padded kz tiles for K=128 st matmuls
            cs = slice(n * SQC, (n + 1) * SQC)
            nc.vector.tensor_scalar_add(kz[0][0:64, cs], ps[2][0:64, :],
                                        bct[2][0:64, :])
            nc.vector.tensor_scalar_add(kz[1][64:128, cs], ps[2][64:128, :],
                                        bct[2][64:128, :])
            nc.vector.tensor_scalar_add(kz[2][64:128, cs], ps[3][64:128, :],
                                        bct[3][64:128, :])

        # ---------------- Phase C: transpose k,v to natural ---------------
        v_sb = []
        for s in range(STILES):
            c0 = s * 128
            kp = tp_ps.tile([128, 192], F32R, tag="kp")
            vp = tp_ps.tile([128, 192], F32R, tag="vp")
            # k: k0k1 = t2 full, k2 = t3[64:128]
            nc.tensor.transpose(kp[:, 0:128], qkvt[2][:, c0:c0 + 128], ident[:])
            nc.tensor.transpose(kp[:, 128:192], qkvt[3][64:128, c0:c0 + 128],
                                ident[64:128, 64:128])
            # v: v0 = t1[0:64], v1 = t3[0:64], v2 = t4[0:64]
            nc.tensor.transpose(vp[:, 0:64], qkvt[1][0:64, c0:c0 + 128],
                                ident[0:64, 0:64])
            nc.tensor.transpose(vp[:, 64:128], qkvt[3][0:64, c0:c0 + 128],
                                ident[0:64, 0:64])
            nc.tensor.transpose(vp[:, 128:192], qkvt[4][0:64, c0:c0 + 128],
                                ident[0:64, 0:64])

            ksb = work.tile([128, HPC, DH], F32, tag="ksb")
            nc.vector.tensor_copy(ksb[:], kp[:].rearrange("p (h d) -> p h d", h=HPC))
            nc.sync.dma_start(
                out=k_out[:, c0:c0 + 128, :].transpose([1, 0, 2]), in_=ksb[:])

            vt = vpool.tile([128, HPC, DH + 1], F32R, tag="vsb")
            nc.vector.tensor_copy(
                vt[:, :, 0:DH], vp[:].rearrange("p (h d) -> p h d", h=HPC))
            nc.vector.tensor_copy(vt[:, :, DH:DH + 1], ones3[:])
            nc.sync.dma_start(
                out=v_out[:, c0:c0 + 128, :].transpose([1, 0, 2]),
                in_=vt[:, :, 0:DH].bitcast(F32))
            v_sb.append(vt)

    if "D" not in env["phases"]:
        return
    # ---------------- Phase D/E: attention + partial proj ---------------
    with ExitStack() as s2:
        st_ps = s2.enter_context(
            tc.tile_pool(name="st_ps", bufs=2, space="PSUM"))
        o_ps = s2.enter_context(
            tc.tile_pool(name="o_ps", bufs=3, space="PSUM"))
        pj_ps = s2.enter_context(
            tc.tile_pool(name="pj_ps", bufs=1, space="PSUM"))

        def emit_proj(c):
            # partial proj for chunk c: aT[:, chunk] = Wp^T @ oT[:, chunk]
            for m in range(6 if "E" in env["phases"] else 0):
                pp = pj_ps.tile([128, SQC], F32, tag="pp")
                nc.tensor.matmul(pp[:], wpt0[:, m * 128:(m + 1) * 128],
                                 ot_a[:, c * SQC:(c + 1) * SQC],
                                 start=True, stop=False)
                nc.tensor.matmul(pp[:], wpt1[:, m * 128:(m + 1) * 128],
                                 ot_b[0:128, c * SQC:(c + 1) * SQC],
                                 start=False, stop=True)
                av = work.tile([128, SQC], F32, tag="av")
                nc.scalar.copy(av[:], pp[:])
                nc.sync.dma_start(
                    out=at_out[m * 128:(m + 1) * 128, c * SQC:(c + 1) * SQC],
                    in_=av[:])

        for c in range(NCHUNK):
            npair = 2 * c + 2
            if c > 0:
                emit_proj(c - 1)
            ops = [o_ps.tile([DH + 1, SQC], F32, tag="op", name=f"op{h}")
                   for h in range(HPC)]
            for pr in range(npair):
                for h in range(HPC):
                    q_rhs = qkvt[Q_AT[h][0]][0:128, c * SQC:(c + 1) * SQC]
                    op = ops[h]
                    stp = st_ps.tile([128, 2 * SQC], F32, tag="stp")
                    nc.tensor.matmul(stp[:, 0:SQC],
                                     kz[h][:, (2 * pr) * 128:(2 * pr + 1) * 128],
                                     q_rhs, start=True, stop=True)
                    nc.tensor.matmul(stp[:, SQC:2 * SQC],
                                     kz[h][:, (2 * pr + 1) * 128:(2 * pr + 2) * 128],
                                     q_rhs, start=True, stop=True)
                    if env["noact"]:
                        p = mask_sb[:, 0, :]
                    else:
                        p = work.tile([128, 2 * SQC], F32R, tag="p", bufs=4)
                        nc.scalar.activation(
                            out=p[:], in_=stp[:],
                            func=mybir.ActivationFunctionType.Exp)
                        jj = pr - 2 * c
                        if jj >= 0:
                            nc.vector.tensor_mul(p[:, 0:SQC], p[:, 0:SQC],
                                                 mask_sb[:, jj, 0:SQC])
                    nc.tensor.matmul(op[:], v_sb[2 * pr][:, h, :], p[:, 0:SQC],
                                     start=(pr == 0), stop=False)
                    if not env["noact"] and jj >= 0:
                        nc.vector.tensor_mul(p[:, SQC:2 * SQC], p[:, SQC:2 * SQC],
                                             mask_sb[:, jj, SQC:2 * SQC])
                    nc.tensor.matmul(op[:], v_sb[2 * pr + 1][:, h, :],
                                     p[:, SQC:2 * SQC],
                                     start=False, stop=(pr == npair - 1))
            for h in range(HPC):
                # normalize: oT = o_un * (1/Z) broadcast over partitions
                op = ops[h]
                rz = small.tile([1, SQC], F32, tag="rz")
                nc.vector.reciprocal(rz[:], op[DH:DH + 1, :])
                bz = work.tile([64, SQC], F32, tag="bz")
                nc.gpsimd.partition_broadcast(bz[:], rz[:])
                dst = (ot_a[h * DH:(h + 1) * DH, c * SQC:(c + 1) * SQC]
                       if h < 2 else ot_b[0:64, c * SQC:(c + 1) * SQC])
                nc.vector.tensor_mul(dst, op[0:DH, :], bz[:])
        emit_proj(NCHUNK - 1)



_NC_CACHE = {}


def _get_nc():
    if "nc" not in _NC_CACHE:
        _NC_CACHE["nc"] = build_nc()
    return _NC_CACHE["nc"]


def make_in_maps(x, w_attn, b_attn, lora_A, lora_B, w_proj):
    x = np.asarray(x, dtype=np.float32)
    w_attn = np.asarray(w_attn, dtype=np.float32)
    b_attn = np.asarray(b_attn, dtype=np.float32)
    A = np.asarray(lora_A, dtype=np.float32).reshape(3, 8, NX)
    Bm = np.asarray(lora_B, dtype=np.float32).reshape(3, NX, 8)
    w_proj = np.asarray(w_proj, dtype=np.float32)

    delta = np.concatenate(
        [(Bm[c].astype(np.float64) @ A[c].astype(np.float64)).T for c in range(3)],
        axis=1)
    weff = (w_attn.astype(np.float64) + 16.0 * delta).astype(np.float32)
    beff = b_attn.astype(np.float32).copy()
    weff[:, :NX] *= 0.125  # fold 1/sqrt(dh) into q (exact)
    beff[:NX] *= 0.125

    r = np.arange(128)[:, None]
    cc = np.arange(SQC)[None, :]
    # paired masks: masks[jj][:, 0:512] = tile 2jj, [:, 512:1024] = tile 2jj+1
    m4 = [(r + 128 * j <= cc).astype(np.float32) for j in range(4)]
    masks = np.stack([np.concatenate([m4[0], m4[1]], axis=1),
                      np.concatenate([m4[2], m4[3]], axis=1)])

    in_maps = []
    for core in range(8):
        b, g = divmod(core, 4)
        sl = slice(192 * g, 192 * (g + 1))
        q = weff[:, sl]
        k = weff[:, NX:][:, sl]
        v = weff[:, 2 * NX:][:, sl]
        bq = beff[sl]
        bk = beff[NX:][sl]
        bv = beff[2 * NX:][sl]
        # row order [q0 q1 | v0 q2 | k0 k1 | v1 k2 | v2]
        wcore = np.ascontiguousarray(np.concatenate(
            [q[:, 0:128], v[:, 0:64], q[:, 128:192],
             k[:, 0:128], v[:, 64:128], k[:, 128:192], v[:, 128:192]], axis=1))
        bcv = np.ascontiguousarray(np.concatenate(
            [bq[0:128], bv[0:64], bq[128:192],
             bk[0:128], bv[64:128], bk[128:192], bv[128:192]])[:, None])
        in_maps.append({
            "zeros": np.zeros((64, S), dtype=np.float32),
            "xt": round_f32r(x[b].T),
            "wc": round_f32r(wcore),
            "bc": bcv,
            "wp": round_f32r(w_proj[sl, :]),
            "masks": masks,
        })
    return in_maps


def kernel(x, w_attn, b_attn, lora_A, lora_B, w_proj, b_proj):
    nc = _get_nc()
    in_maps = make_in_maps(x, w_attn, b_attn, lora_A, lora_B, w_proj)
    res = bass_utils.run_bass_kernel_spmd(nc, in_maps, core_ids=list(range(8)))

    b_proj = np.asarray(b_proj, dtype=np.float32)
    a = np.zeros((2, S, NX), dtype=np.float32)
    present = np.zeros((2, 2, 12, S, DH), dtype=np.float32)
    for core in range(8):
        b, g = divmod(core, 4)
        r = res.results[core]
        a[b] += r["at_out"].T
        present[0, b, 3 * g:3 * g + 3] = r["k_out"]
        present[1, b, 3 * g:3 * g + 3] = r["v_out"]
    a += b_proj
    return a, present
